# revision 5
# baseline (speedup 1.0000x reference)
"""Distributed Trainium2 Bass kernel for the AttGNN (2x GATConv branches +
global mean pool + fc) problem.

Strategy (8 NeuronCores):
  - Branch parallel: cores 0-3 run branch (x, adj), cores 4-7 run
    (wt_x, wt_adj) concurrently (weights shared, SPMD graph identical).
  - Within a 4-core group: destination-node sharding. Nodes are remapped on
    the host into 40 "windows" of 128 slots per core (127 real + 1 pad slot),
    LPT-balanced by in-degree so each window has a near-equal edge count.
  - Node phase: each core computes h = x@W (bf16 matmul), plus the per-node
    attention scalars a_src/a_dst (folded into W on the host), packs
    [h | a_src | a_dst] into 1280B rows, and AllGathers the packed table.
  - Edge phase: per window, dma_gather fetches packed rows by edge src
    (1280B) and the a-cols by edge dst (256B).  exp(leaky(a_src+a_dst)) is
    computed vectorized; the segment softmax numerator and denominator are
    accumulated with a one-hot scatter matmul (one-hot built on-device from
    per-edge dst-slot values); normalization is deferred and fused into a
    Prelu activation with per-partition scale = 1/denom.
  - Masked column-sum matmul accumulates the global mean pool; a tiny
    8-core AllGather + on-device fc1 finishes both branches redundantly.

Host-side work is restricted to topology preprocessing (edge partitioning,
index/layout construction) and weight folding; all FLOPs on float data
happen on device.
"""

import sys

if "/opt/trn_rl_repo" not in sys.path:
    sys.path.insert(0, "/opt/trn_rl_repo")

import heapq

import numpy as np
import ml_dtypes

BF16 = ml_dtypes.bfloat16

N = 20000
F = 512
HEADS = 4
HC = 512
NCORES = 8
GROUP = 4
NW = 40                 # windows per core
WSLOT = 128             # slots per window (127 real + 1 pad)
CAP = 127
SLOTS = NW * WSLOT      # 5120 slots per core
GSLOTS = GROUP * SLOTS  # 20480 slots per branch group
ROWB = 640              # packed row length in bf16 elems (1280 B)
NEG_ATT = 0.2
NEG_ACT = 0.01


# --------------------------------------------------------------------------
# host-side topology preprocessing
# --------------------------------------------------------------------------

def _lpt_assign(indeg):
    """Assign nodes to GROUP*NW bins balanced by in-degree, <=CAP nodes/bin."""
    nbins = GROUP * NW
    order = np.argsort(-indeg, kind="stable")
    load = np.zeros(nbins, dtype=np.int64)
    nnodes = np.zeros(nbins, dtype=np.int32)
    heap = [(0, b) for b in range(nbins)]
    heapq.heapify(heap)
    core_of = np.empty(N, np.int32)
    win_of = np.empty(N, np.int32)
    pos_of = np.empty(N, np.int32)
    for n in order:
        while True:
            l, b = heapq.heappop(heap)
            if nnodes[b] >= CAP or l != load[b]:
                continue
            break
        core_of[n] = b // NW
        win_of[n] = b % NW
        pos_of[n] = nnodes[b]
        nnodes[b] += 1
        load[b] += int(indeg[n])
        if nnodes[b] < CAP:
            heapq.heappush(heap, (int(load[b]), b))
    return core_of, win_of, pos_of, nnodes


def _prep_branch(x, adj):
    src0 = np.asarray(adj[0], dtype=np.int64)
    dst0 = np.asarray(adj[1], dtype=np.int64)
    loops = np.arange(N, dtype=np.int64)
    src = np.concatenate([src0, loops])
    dst = np.concatenate([dst0, loops])
    indeg = np.bincount(dst, minlength=N)
    core_of, win_of, pos_of, nnodes = _lpt_assign(indeg)
    slot_of = win_of * WSLOT + pos_of
    gslot_of = core_of * SLOTS + slot_of
    ebin = core_of[dst] * NW + win_of[dst]
    cnt = np.bincount(ebin, minlength=GROUP * NW)
    tu = int(np.ceil(cnt.max() / 128.0))
    return dict(
        x=np.asarray(x, dtype=np.float32),
        src=src, dst=dst, ebin=ebin,
        core_of=core_of, slot_of=slot_of, gslot_of=gslot_of,
        pos_of=pos_of, nnodes=nnodes, tu=tu,
    )


def _wrap_idx(a):
    """[M] -> dma_gather idx layout [128, M//16] (wrapped, replicated x8)."""
    return np.ascontiguousarray(np.tile(a.reshape(-1, 16).T, (8, 1)).astype(np.int16))


def _finalize_branch(pb, tu):
    nbins = GROUP * NW
    ew = tu * 128
    order = np.argsort(pb["ebin"], kind="stable")
    sb = pb["ebin"][order]
    ssrc = pb["gslot_of"][pb["src"][order]]
    sslot = pb["pos_of"][pb["dst"][order]]
    sgdst = pb["gslot_of"][pb["dst"][order]]
    counts = np.bincount(sb, minlength=nbins)
    starts = np.zeros(nbins, np.int64)
    np.cumsum(counts[:-1], out=starts[1:])
    within = np.arange(len(sb)) - starts[sb]

    srcpad = np.zeros((nbins, ew), np.int64)
    slotpad = np.full((nbins, ew), CAP, np.int64)
    binidx = np.arange(nbins)
    base = (binidx // NW) * SLOTS + (binidx % NW) * WSLOT
    didxpad = np.broadcast_to((base + CAP)[:, None], (nbins, ew)).copy()
    srcpad[sb, within] = ssrc
    slotpad[sb, within] = sslot
    didxpad[sb, within] = sgdst

    per_core = []
    for c in range(GROUP):
        s = srcpad[c * NW:(c + 1) * NW].reshape(-1)
        d = didxpad[c * NW:(c + 1) * NW].reshape(-1)
        sv = slotpad[c * NW:(c + 1) * NW]
        sidx = _wrap_idx(s)
        didx = _wrap_idx(d)
        slv = np.ascontiguousarray(
            sv.reshape(NW, tu, 128).transpose(2, 0, 1).reshape(128, NW * tu)
        ).astype(BF16)
        cmask = np.zeros((128, NW), np.float32)
        for w in range(NW):
            cmask[: pb["nnodes"][c * NW + w], w] = 1.0
        nodes_c = np.where(pb["core_of"] == c)[0]
        xcore = np.zeros((SLOTS, F), np.float32)
        xcore[pb["slot_of"][nodes_c]] = pb["x"][nodes_c]
        xt = np.ascontiguousarray(xcore.T)
        per_core.append(dict(xt=xt, sidx=sidx, didx=didx, slv=slv,
                             cmk=cmask.astype(BF16)))
    return per_core


# --------------------------------------------------------------------------
# device graph
# --------------------------------------------------------------------------

_BUILD_CACHE = {}


def _build(tu, use_bias):
    key = (tu, use_bias)
    if key in _BUILD_CACHE:
        return _BUILD_CACHE[key]

    from contextlib import ExitStack

    import concourse.bass as bass
    import concourse.mybir as mybir
    import concourse.bacc as bacc
    from concourse import tile
    from concourse.alu_op_type import AluOpType as AO

    f32 = mybir.dt.float32
    bf16 = mybir.dt.bfloat16
    i16 = mybir.dt.int16
    AF = mybir.ActivationFunctionType

    ew = tu * 128

    def bc(ap, pos, n):
        """Insert a broadcast (step 0) dim of extent n at position pos."""
        new = list(ap.ap)
        new.insert(pos, [0, n])
        return bass.AP(ap.tensor, ap.offset, new)

    nc = bacc.Bacc("TRN2", target_bir_lowering=False, debug=False)
    xt = nc.declare_dram_parameter("xt", [F, SLOTS], f32, isOutput=False)
    wm = nc.declare_dram_parameter("wm", [F, 520], f32, isOutput=False)
    f1t = nc.declare_dram_parameter("f1t", [F, F], f32, isOutput=False)
    f1b = nc.declare_dram_parameter("f1b", [128, 4], f32, isOutput=False)
    iot = nc.declare_dram_parameter("iot", [128, 128], bf16, isOutput=False)
    sidx = nc.declare_dram_parameter("sidx", [128, NW * tu * 8], i16, isOutput=False)
    didx = nc.declare_dram_parameter("didx", [128, NW * tu * 8], i16, isOutput=False)
    slv = nc.declare_dram_parameter("slv", [128, NW * tu], bf16, isOutput=False)
    cmk = nc.declare_dram_parameter("cmk", [128, NW], bf16, isOutput=False)
    if use_bias:
        brp = nc.declare_dram_parameter("brp", [128, 512], f32, isOutput=False)
    outp = nc.declare_dram_parameter("out", [128, 12], f32, isOutput=True)

    with tile.TileContext(nc) as tc, ExitStack() as ctx:
        dram = ctx.enter_context(tc.tile_pool(name="dram", bufs=1, space="DRAM"))
        hpl = dram.tile([SLOTS, ROWB], bf16)
        # NOTE: Shared output needs >4-core groups; 4-core AG must be Local.
        hpa = dram.tile([GSLOTS, ROWB], bf16)
        gsl = dram.tile([1, 512], f32)
        gsa = dram.tile([8, 512], f32, addr_space="Shared")

        const = ctx.enter_context(tc.tile_pool(name="const", bufs=1))
        wkb = const.tile([128, 4, 520], bf16)
        f1tb = const.tile([128, 4, 512], bf16)
        f1bs = const.tile([128, 4], f32)
        iots = const.tile([128, 128], bf16)
        slvs = const.tile([128, NW * tu], bf16)
        cmks = const.tile([128, NW], bf16)
        sidxs = const.tile([128, NW * tu * 8], i16)
        didxs = const.tile([128, NW * tu * 8], i16)
        al_att = const.tile([128, 1], f32)
        al_act = const.tile([128, 1], f32)
        nc.any.memset(al_att[:, :], NEG_ATT)
        nc.any.memset(al_act[:, :], NEG_ACT)
        if use_bias:
            brps = const.tile([128, 512], f32)
            nc.sync.dma_start(brps[:, :], brp[:, :])

        with tc.tile_pool(name="stage", bufs=2) as stage:
            ws = stage.tile([128, 4, 520], f32, tag="st")
            for k in range(4):
                nc.sync.dma_start(ws[:, k, :], wm[k * 128:(k + 1) * 128, :])
            nc.vector.tensor_copy(wkb[:, :, :], ws[:, :, :])
            fs = stage.tile([128, 4, 512], f32, tag="st")
            for k in range(4):
                nc.sync.dma_start(fs[:, k, :], f1t[k * 128:(k + 1) * 128, :])
            nc.vector.tensor_copy(f1tb[:, :, :], fs[:, :, :])
        nc.sync.dma_start(f1bs[:, :], f1b[:, :])
        nc.sync.dma_start(iots[:, :], iot[:, :])
        nc.sync.dma_start(slvs[:, :], slv[:, :])
        nc.sync.dma_start(cmks[:, :], cmk[:, :])
        nc.sync.dma_start(sidxs[:, :], sidx[:, :])
        nc.sync.dma_start(didxs[:, :], didx[:, :])

        # ---- node phase: h = x @ W, a = x @ [Wa_src | Wa_dst] ----
        with tc.tile_pool(name="xp", bufs=3) as xpool, \
             tc.tile_pool(name="hp", bufs=3) as hpool, \
             tc.tile_pool(name="p1ps", bufs=2, space="PSUM") as p1ps:
            for ch in range(NW):
                xs = xpool.tile([128, 4, 128], f32, tag="xs")
                for k in range(4):
                    nc.sync.dma_start(
                        xs[:, k, :],
                        xt[k * 128:(k + 1) * 128, ch * 128:(ch + 1) * 128])
                xb = xpool.tile([128, 4, 128], bf16, tag="xb")
                nc.vector.tensor_copy(xb[:, :, :], xs[:, :, :])
                ph = p1ps.tile([128, 512], f32, tag="ph")
                pa = p1ps.tile([128, 8], f32, tag="pa")
                for k in range(4):
                    nc.tensor.matmul(ph[:, :], xb[:, k, :], wkb[:, k, 0:512],
                                     start=(k == 0), stop=(k == 3))
                    nc.tensor.matmul(pa[:, :], xb[:, k, :], wkb[:, k, 512:520],
                                     start=(k == 0), stop=(k == 3))
                hp = hpool.tile([128, ROWB], bf16, tag="hp")
                nc.vector.tensor_copy(hp[:, 0:512], ph[:, :])
                nc.vector.tensor_copy(hp[:, 512:520], pa[:, :])
                nc.sync.dma_start(hpl[ch * 128:(ch + 1) * 128, :], hp[:, :])

        # ---- AllGather the packed node table within each branch group ----
        nc.gpsimd.collective_compute(
            "AllGather", AO.bypass,
            replica_groups=[[0, 1, 2, 3], [4, 5, 6, 7]],
            ins=[hpl[:, :]], outs=[hpa[:, :]])

        # ---- edge phase ----
        with tc.tile_pool(name="gp", bufs=2) as gpool, \
             tc.tile_pool(name="sp", bufs=2) as spool, \
             tc.tile_pool(name="ap2", bufs=3) as apool, \
             tc.tile_pool(name="p3ps", bufs=2, space="PSUM") as p3ps, \
             tc.tile_pool(name="pcps", bufs=1, space="PSUM") as pcps:
            pc_ = pcps.tile([1, 512], f32, tag="pC")
            for w in range(NW):
                hpt = gpool.tile([128, tu, ROWB], bf16, tag="hpt")
                nc.gpsimd.dma_gather(
                    hpt[:, :, :], hpa[:, :],
                    sidxs[:, w * tu * 8:(w + 1) * tu * 8], ew, ew, ROWB,
                    single_packet=False)
                adt = gpool.tile([128, tu, 128], bf16, tag="adt")
                nc.gpsimd.dma_gather(
                    adt[:, :, :], hpa[:, 512:640],
                    didxs[:, w * tu * 8:(w + 1) * tu * 8], ew, ew, 128,
                    elem_step=ROWB, single_packet=False)

                es = apool.tile([128, tu, 4], f32, tag="es")
                nc.vector.tensor_tensor(es[:, :, :], hpt[:, :, 512:516],
                                        adt[:, :, 4:8], AO.add)
                el = apool.tile([128, tu, 4], f32, tag="el")
                nc.scalar.activation(el[:, :, :], es[:, :, :], AF.Prelu,
                                     alpha=al_att[:, :])
                ex = apool.tile([128, tu, 4], bf16, tag="ex")
                nc.scalar.activation(ex[:, :, :], el[:, :, :], AF.Exp)

                sg = spool.tile([128, tu, 128], bf16, tag="sg")
                sl = slvs[:, w * tu:(w + 1) * tu]
                nc.vector.tensor_tensor(
                    sg[:, :, :], bc(iots[:, :], 1, tu), bc(sl, 2, 128),
                    AO.is_equal)

                msg = spool.tile([128, tu, 512], bf16, tag="msg")
                h4 = hpt[:, :, 0:512].rearrange("p t (h c) -> p t h c", c=128)
                m4 = msg[:, :, :].rearrange("p t (h c) -> p t h c", c=128)
                nc.vector.tensor_tensor(m4, h4, bc(ex[:, :, :], 3, 128), AO.mult)

                pO = p3ps.tile([128, 512], f32, tag="pO")
                pD = p3ps.tile([128, 4], f32, tag="pD")
                for t in range(tu):
                    nc.tensor.matmul(pO[:, :], sg[:, t, :], msg[:, t, :],
                                     start=(t == 0), stop=(t == tu - 1))
                    nc.tensor.matmul(pD[:, :], sg[:, t, :], ex[:, t, :],
                                     start=(t == 0), stop=(t == tu - 1))

                de = apool.tile([128, 4], f32, tag="de")
                nc.vector.tensor_scalar_add(de[:, :], pD[:, :], 1e-16)
                rc = apool.tile([128, 4], f32, tag="rc")
                nc.vector.reciprocal(rc[:, :], de[:, :])

                ab = spool.tile([128, 512], bf16, tag="ab")
                if use_bias:
                    nb = spool.tile([128, 512], f32, tag="nb")
                    for h in range(4):
                        nc.vector.tensor_scalar(
                            nb[:, h * 128:(h + 1) * 128],
                            pO[:, h * 128:(h + 1) * 128],
                            rc[:, h:h + 1], None, AO.mult)
                    nc.vector.tensor_tensor(nb[:, :], nb[:, :], brps[:, :], AO.add)
                    nc.scalar.activation(ab[:, :], nb[:, :], AF.Prelu,
                                         alpha=al_act[:, :])
                else:
                    for h in range(4):
                        nc.scalar.activation(
                            ab[:, h * 128:(h + 1) * 128],
                            pO[:, h * 128:(h + 1) * 128],
                            AF.Prelu, scale=rc[:, h:h + 1], alpha=al_act[:, :])

                nc.tensor.matmul(pc_[:, :], cmks[:, w:w + 1], ab[:, :],
                                 start=(w == 0), stop=(w == NW - 1),
                                 skip_group_check=True)

            # ---- global mean + fc1 (redundant on every core) ----
            with tc.tile_pool(name="p4", bufs=1) as p4:
                gs = p4.tile([1, 512], f32)
                nc.vector.tensor_scalar(gs[:, :], pc_[:, :], 1.0 / N, None, AO.mult)
                nc.sync.dma_start(gsl[:, :], gs[:, :])
                nc.gpsimd.collective_compute(
                    "AllGather", AO.bypass,
                    replica_groups=[[0, 1, 2, 3, 4, 5, 6, 7]],
                    ins=[gsl[:, :]], outs=[gsa[:, :]])
                gtr = p4.tile([128, 4, 8], f32)
                for r in range(8):
                    nc.sync.dma_start(
                        gtr[:, :, r],
                        gsa[r:r + 1, :].rearrange("o (c p) -> (o p) c", p=128))
                gt = p4.tile([128, 4, 2], f32)
                nc.vector.reduce_sum(
                    gt[:, :, :],
                    gtr[:, :, :].rearrange("p c (g k) -> p c g k", k=4),
                    mybir.AxisListType.X)
                gtb = p4.tile([128, 4, 2], bf16)
                nc.vector.tensor_copy(gtb[:, :, :], gt[:, :, :])
                pF = pcps.tile([128, 8], f32, tag="pF")
                for m in range(4):
                    for k in range(4):
                        nc.tensor.matmul(
                            pF[:, m * 2:(m + 1) * 2],
                            f1tb[:, k, m * 128:(m + 1) * 128], gtb[:, k, :],
                            start=(k == 0), stop=(k == 3),
                            skip_group_check=True)
                fo = p4.tile([128, 4, 3], f32)
                for m in range(4):
                    nc.scalar.activation(fo[:, m, 0:2], pF[:, m * 2:(m + 1) * 2],
                                         AF.Prelu, bias=f1bs[:, m:m + 1],
                                         alpha=al_act[:, :])
                nc.vector.tensor_tensor(fo[:, :, 2:3], fo[:, :, 0:1],
                                        fo[:, :, 1:2], AO.subtract)
                nc.sync.dma_start(outp[:, :], fo[:, :, :])

    nc.compile()
    _BUILD_CACHE[key] = nc
    return nc


# --------------------------------------------------------------------------
# entry point
# --------------------------------------------------------------------------

def kernel(x, adj, wt_x, wt_adj, W, att_src, att_dst, bias, fc1_w, fc1_b):
    from concourse.bass_utils import run_bass_kernel_spmd

    x = np.asarray(x, np.float32)
    wt_x = np.asarray(wt_x, np.float32)
    adj = np.asarray(adj)
    wt_adj = np.asarray(wt_adj)
    W = np.asarray(W, np.float32)
    att_src = np.asarray(att_src, np.float32)
    att_dst = np.asarray(att_dst, np.float32)
    bias = np.asarray(bias, np.float32)
    fc1_w = np.asarray(fc1_w, np.float32)
    fc1_b = np.asarray(fc1_b, np.float32)

    pba = _prep_branch(x, adj)
    pbb = _prep_branch(wt_x, wt_adj)
    tu = max(pba["tu"], pbb["tu"])
    pca = _finalize_branch(pba, tu)
    pcb = _finalize_branch(pbb, tu)

    wa_s = np.einsum("fhc,hc->fh", W.reshape(F, HEADS, 128), att_src)
    wa_d = np.einsum("fhc,hc->fh", W.reshape(F, HEADS, 128), att_dst)
    wmat = np.ascontiguousarray(
        np.concatenate([W, wa_s, wa_d], axis=1).astype(np.float32))
    f1br = np.ascontiguousarray(fc1_b.reshape(4, 128).T.astype(np.float32))
    iotn = np.ascontiguousarray(
        np.tile(np.arange(128, dtype=np.float32), (128, 1)).astype(BF16))
    use_bias = bool(np.any(bias != 0.0))

    nc = _build(tu, use_bias)

    in_maps = []
    for core in range(NCORES):
        pc = (pca if core < GROUP else pcb)[core % GROUP]
        m = dict(xt=pc["xt"], wm=wmat, f1t=fc1_w, f1b=f1br, iot=iotn,
                 sidx=pc["sidx"], didx=pc["didx"], slv=pc["slv"],
                 cmk=pc["cmk"])
        if use_bias:
            m["brp"] = np.ascontiguousarray(
                np.tile(bias[None, :], (128, 1)).astype(np.float32))
        in_maps.append(m)

    trace = bool(int(__import__("os").environ.get("GNN_TRACE", "0")))
    res = run_bass_kernel_spmd(nc, in_maps, core_ids=list(range(NCORES)),
                               trace=trace)
    kernel.last_exec_time_ns = res.exec_time_ns
    o = np.asarray(res.results[0]["out"]).reshape(128, 4, 3)
    o2 = o.transpose(1, 0, 2).reshape(512, 3)
    return np.ascontiguousarray(
        np.concatenate([o2[:, 0], o2[:, 1], o2[:, 2]])[None, :]).astype(np.float32)


# revision 14
# speedup vs baseline: 2.0687x; 2.0687x over previous
"""Distributed Trainium2 Bass kernel for the AttGNN (2x GATConv branches +
global mean pool + fc) problem.

Strategy (8 NeuronCores):
  - Branch parallel: cores 0-3 run branch (x, adj), cores 4-7 run
    (wt_x, wt_adj) concurrently (weights shared, SPMD graph identical).
  - Within a 4-core group: destination-node sharding. Nodes are remapped on
    the host into 40 "windows" of 128 slots per core (127 real + 1 pad slot),
    LPT-balanced by in-degree so each window has a near-equal edge count.
  - Node phase: each core computes h = x@W (bf16 matmul), plus the per-node
    attention scalars a_src/a_dst (folded into W on the host), packs
    [h | a_src | a_dst] into 1280B rows, and AllGathers the packed table.
  - Edge phase: per window, dma_gather fetches packed rows by edge src
    (1280B) and the a-cols by edge dst (256B).  exp(leaky(a_src+a_dst)) is
    computed vectorized; the segment softmax numerator and denominator are
    accumulated with a one-hot scatter matmul (one-hot built on-device from
    per-edge dst-slot values); normalization is deferred and fused into a
    Prelu activation with per-partition scale = 1/denom.
  - Masked column-sum matmul accumulates the global mean pool; a tiny
    8-core AllGather + on-device fc1 finishes both branches redundantly.

Host-side work is restricted to topology preprocessing (edge partitioning,
index/layout construction) and weight folding; all FLOPs on float data
happen on device.
"""

import sys

if "/opt/trn_rl_repo" not in sys.path:
    sys.path.insert(0, "/opt/trn_rl_repo")

import heapq

import numpy as np
import ml_dtypes

BF16 = ml_dtypes.bfloat16

N = 20000
F = 512
HEADS = 4
HC = 512
NCORES = 8
GROUP = 4
NW = 40                 # windows per core
WSLOT = 128             # slots per window (127 real + 1 pad)
CAP = 127
SLOTS = NW * WSLOT      # 5120 slots per core
GSLOTS = GROUP * SLOTS  # 20480 slots per branch group
AGC = 4                 # AllGather chunks (overlap with node phase)
ROWB = 768              # packed row bytes: h fp8[512] + a_src/a_dst bf16 + pad
NEG_ATT = 0.2
NEG_ACT = 0.01


# --------------------------------------------------------------------------
# host-side topology preprocessing
# --------------------------------------------------------------------------

def _lpt_assign(indeg):
    """Assign nodes to GROUP*NW bins balanced by in-degree, <=CAP nodes/bin."""
    nbins = GROUP * NW
    order = np.argsort(-indeg, kind="stable")
    load = np.zeros(nbins, dtype=np.int64)
    nnodes = np.zeros(nbins, dtype=np.int32)
    heap = [(0, b) for b in range(nbins)]
    heapq.heapify(heap)
    core_of = np.empty(N, np.int32)
    win_of = np.empty(N, np.int32)
    pos_of = np.empty(N, np.int32)
    for n in order:
        while True:
            l, b = heapq.heappop(heap)
            if nnodes[b] >= CAP or l != load[b]:
                continue
            break
        core_of[n] = b // NW
        win_of[n] = b % NW
        pos_of[n] = nnodes[b]
        nnodes[b] += 1
        load[b] += int(indeg[n])
        if nnodes[b] < CAP:
            heapq.heappush(heap, (int(load[b]), b))
    return core_of, win_of, pos_of, nnodes


def _prep_branch(x, adj):
    src0 = np.asarray(adj[0], dtype=np.int64)
    dst0 = np.asarray(adj[1], dtype=np.int64)
    loops = np.arange(N, dtype=np.int64)
    src = np.concatenate([src0, loops])
    dst = np.concatenate([dst0, loops])
    indeg = np.bincount(dst, minlength=N)
    core_of, win_of, pos_of, nnodes = _lpt_assign(indeg)
    slot_of = win_of * WSLOT + pos_of
    # chunked AllGather layout: chunk k holds ranks' rows [k*1280,(k+1)*1280)
    chunk = slot_of // (SLOTS // AGC)
    gslot_of = (chunk * GROUP + core_of) * (SLOTS // AGC) \
        + (slot_of - chunk * (SLOTS // AGC))
    ebin = core_of[dst] * NW + win_of[dst]
    cnt = np.bincount(ebin, minlength=GROUP * NW)
    tu = int(np.ceil(cnt.max() / 128.0))
    return dict(
        x=np.asarray(x, dtype=np.float32),
        src=src, dst=dst, ebin=ebin,
        core_of=core_of, slot_of=slot_of, gslot_of=gslot_of,
        pos_of=pos_of, nnodes=nnodes, tu=tu,
    )


def _wrap_idx(a):
    """[M] -> dma_gather idx layout [128, M//16] (wrapped, replicated x8)."""
    return np.ascontiguousarray(np.tile(a.reshape(-1, 16).T, (8, 1)).astype(np.int16))


def _finalize_branch(pb, tu):
    nbins = GROUP * NW
    ew = tu * 128
    order = np.argsort(pb["ebin"], kind="stable")
    sb = pb["ebin"][order]
    ssrc = pb["gslot_of"][pb["src"][order]]
    sslot = pb["pos_of"][pb["dst"][order]]
    counts = np.bincount(sb, minlength=nbins)
    starts = np.zeros(nbins, np.int64)
    np.cumsum(counts[:-1], out=starts[1:])
    within = np.arange(len(sb)) - starts[sb]

    srcpad = np.zeros((nbins, ew), np.int64)
    slotpad = np.full((nbins, ew), CAP, np.int64)
    srcpad[sb, within] = ssrc
    slotpad[sb, within] = sslot

    FP8 = ml_dtypes.float8_e4m3
    per_core = []
    for c in range(GROUP):
        s = srcpad[c * NW:(c + 1) * NW].reshape(-1)
        sv = slotpad[c * NW:(c + 1) * NW].reshape(NW, tu, 128)
        sidx = _wrap_idx(s)
        onehot = sv[:, :, :, None] == np.arange(128)[None, None, None, :]
        # sfp[w*128+e, t*128+slot]: scatter one-hot (edges on partitions)
        sfp = np.ascontiguousarray(
            onehot.transpose(0, 2, 1, 3).reshape(NW * 128, tu * 128)
        ).astype(FP8)
        # stp[w*128+slot, t*128+e]: expand one-hot (slots on partitions)
        stp = np.ascontiguousarray(
            onehot.transpose(0, 3, 1, 2).reshape(NW * 128, tu * 128)
        ).astype(FP8)
        cmask = np.zeros((128, NW), np.float32)
        for w in range(NW):
            cmask[: pb["nnodes"][c * NW + w], w] = 1.0
        nodes_c = np.where(pb["core_of"] == c)[0]
        xcore = np.zeros((SLOTS, F), np.float32)
        xcore[pb["slot_of"][nodes_c]] = pb["x"][nodes_c]
        xt = np.ascontiguousarray(xcore.T)
        per_core.append(dict(xt=xt, sidx=sidx, sfp=sfp, stp=stp,
                             cmk=cmask.astype(BF16)))
    return per_core


# --------------------------------------------------------------------------
# device graph
# --------------------------------------------------------------------------

_BUILD_CACHE = {}


def _build(tu, use_bias):
    key = (tu, use_bias)
    if key in _BUILD_CACHE:
        return _BUILD_CACHE[key]

    from contextlib import ExitStack

    import concourse.bass as bass
    import concourse.mybir as mybir
    import concourse.bacc as bacc
    from concourse import tile
    from concourse.alu_op_type import AluOpType as AO

    f32 = mybir.dt.float32
    bf16 = mybir.dt.bfloat16
    i16 = mybir.dt.int16
    AF = mybir.ActivationFunctionType

    ew = tu * 128

    def bc(ap, pos, n):
        """Insert a broadcast (step 0) dim of extent n at position pos."""
        new = list(ap.ap)
        new.insert(pos, [0, n])
        return bass.AP(ap.tensor, ap.offset, new)

    fp8 = mybir.dt.float8e4
    nc = bacc.Bacc("TRN2", target_bir_lowering=False, debug=False)
    xt = nc.declare_dram_parameter("xt", [F, SLOTS], f32, isOutput=False)
    wm = nc.declare_dram_parameter("wm", [F, 520], f32, isOutput=False)
    f1t = nc.declare_dram_parameter("f1t", [F, F], f32, isOutput=False)
    f1b = nc.declare_dram_parameter("f1b", [128, 4], f32, isOutput=False)
    sidx = nc.declare_dram_parameter("sidx", [128, NW * tu * 8], i16, isOutput=False)
    sfp = nc.declare_dram_parameter("sfp", [NW * 128, tu * 128], fp8, isOutput=False)
    stp = nc.declare_dram_parameter("stp", [NW * 128, tu * 128], fp8, isOutput=False)
    cmk = nc.declare_dram_parameter("cmk", [128, NW], bf16, isOutput=False)
    if use_bias:
        brp = nc.declare_dram_parameter("brp", [128, 512], f32, isOutput=False)
    outp = nc.declare_dram_parameter("out", [128, 12], f32, isOutput=True)

    with tile.TileContext(nc) as tc, ExitStack() as ctx:
        dram = ctx.enter_context(tc.tile_pool(name="dram", bufs=1, space="DRAM"))
        csl = SLOTS // AGC
        hpls = [dram.tile([csl, ROWB], fp8, name=f"hpl{k}") for k in range(AGC)]
        # NOTE: Shared output needs >4-core groups; 4-core AG must be Local.
        hpa = dram.tile([GSLOTS, ROWB], fp8)
        gsl = dram.tile([1, 512], f32)
        gsa = dram.tile([8, 512], f32, addr_space="Shared")

        const = ctx.enter_context(tc.tile_pool(name="const", bufs=1))
        wkb = const.tile([128, 4, 520], bf16)
        f1tb = const.tile([128, 4, 512], bf16)
        f1bs = const.tile([128, 4], f32)
        cmks = const.tile([128, NW], bf16)
        sidxs = const.tile([128, NW * tu * 8], i16)
        adl = const.tile([128, NW, 4], bf16)
        al_att = const.tile([128, 1], f32)
        al_act = const.tile([128, 1], f32)
        nc.any.memset(al_att[:, :], NEG_ATT)
        nc.any.memset(al_act[:, :], NEG_ACT)
        if use_bias:
            brps = const.tile([128, 512], f32)
            nc.sync.dma_start(brps[:, :], brp[:, :])

        with tc.tile_pool(name="stage", bufs=2) as stage:
            ws = stage.tile([128, 4, 520], f32, tag="st")
            for k in range(4):
                nc.sync.dma_start(ws[:, k, :], wm[k * 128:(k + 1) * 128, :])
            nc.vector.tensor_copy(wkb[:, :, :], ws[:, :, :])
            fs = stage.tile([128, 4, 512], f32, tag="st")
            for k in range(4):
                nc.sync.dma_start(fs[:, k, :], f1t[k * 128:(k + 1) * 128, :])
            nc.vector.tensor_copy(f1tb[:, :, :], fs[:, :, :])
        nc.sync.dma_start(f1bs[:, :], f1b[:, :])
        nc.sync.dma_start(cmks[:, :], cmk[:, :])
        nc.sync.dma_start(sidxs[:, :], sidx[:, :])

        # ---- node phase: h = x @ W, a = x @ [Wa_src | Wa_dst] ----
        with tc.tile_pool(name="xp", bufs=3) as xpool, \
             tc.tile_pool(name="hp", bufs=3) as hpool, \
             tc.tile_pool(name="p1ps", bufs=2, space="PSUM") as p1ps:
            for ch in range(NW):
                xs = xpool.tile([128, 4, 128], f32, tag="xs")
                nc.sync.dma_start(
                    xs[:, :, :],
                    xt[:, ch * 128:(ch + 1) * 128].rearrange(
                        "(k p) c -> p k c", p=128))
                xb = xpool.tile([128, 4, 128], bf16, tag="xb")
                nc.vector.tensor_copy(xb[:, :, :], xs[:, :, :])
                ph = p1ps.tile([128, 512], f32, tag="ph")
                pa = p1ps.tile([128, 8], f32, tag="pa")
                for k in range(4):
                    nc.tensor.matmul(ph[:, :], xb[:, k, :], wkb[:, k, 0:512],
                                     start=(k == 0), stop=(k == 3))
                    nc.tensor.matmul(pa[:, :], xb[:, k, :], wkb[:, k, 512:520],
                                     start=(k == 0), stop=(k == 3))
                hp = hpool.tile([128, ROWB], fp8, tag="hp")
                nc.vector.tensor_copy(hp[:, 0:512], ph[:, :])
                nc.vector.tensor_copy(hp[:, 512:528].bitcast(bf16), pa[:, :])
                nc.vector.tensor_copy(adl[:, ch, :], pa[:, 4:8])
                k = ch // (NW // AGC)
                r0 = (ch % (NW // AGC)) * 128
                nc.sync.dma_start(hpls[k][r0:r0 + 128, :], hp[:, :])

        # ---- AllGather the packed node table within each branch group ----
        for k in range(AGC):
            nc.gpsimd.collective_compute(
                "AllGather", AO.bypass,
                replica_groups=[[0, 1, 2, 3], [4, 5, 6, 7]],
                ins=[hpls[k][:, :]],
                outs=[hpa[k * GROUP * csl:(k + 1) * GROUP * csl, :]])

        # ---- edge phase ----
        with tc.tile_pool(name="gp", bufs=2) as gpool, \
             tc.tile_pool(name="sp", bufs=2) as spool, \
             tc.tile_pool(name="ap2", bufs=3) as apool, \
             tc.tile_pool(name="p3ps", bufs=2, space="PSUM") as p3ps, \
             tc.tile_pool(name="pcps", bufs=1, space="PSUM") as pcps:
            pc_ = pcps.tile([1, 512], f32, tag="pC")
            for w in range(NW):
                hpt = gpool.tile([128, tu, ROWB], fp8, tag="hpt", bufs=4)
                nc.gpsimd.dma_gather(
                    hpt[:, :, :], hpa[:, :],
                    sidxs[:, w * tu * 8:(w + 1) * tu * 8], ew, ew, ROWB,
                    single_packet=False)
                sg = spool.tile([128, tu, 128], fp8, tag="sg", bufs=3)
                nc.sync.dma_start(sg[:, :, :], sfp[w * 128:(w + 1) * 128, :])
                sT = spool.tile([128, tu, 128], fp8, tag="sT", bufs=3)
                nc.sync.dma_start(sT[:, :, :], stp[w * 128:(w + 1) * 128, :])

                # expand a_dst of this window's slots to its edges via matmul
                pE = p3ps.tile([128, tu * 4], f32, tag="pE")
                for t in range(tu):
                    nc.tensor.matmul(pE[:, t * 4:(t + 1) * 4], sT[:, t, :],
                                     adl[:, w, :], start=True, stop=True,
                                     skip_group_check=True)

                es = apool.tile([128, tu, 4], f32, tag="es")
                nc.vector.tensor_tensor(
                    es[:, :, :], hpt[:, :, 512:520].bitcast(bf16),
                    pE[:, :].rearrange("p (t a) -> p t a", a=4), AO.add)
                el = apool.tile([128, tu, 4], f32, tag="el")
                nc.scalar.activation(el[:, :, :], es[:, :, :], AF.Prelu,
                                     alpha=al_att[:, :])
                ex = apool.tile([128, tu, 4], bf16, tag="ex")
                nc.scalar.activation(ex[:, :, :], el[:, :, :], AF.Exp)

                # msg = exp(el) broadcast over feat (ACT), then *h in place
                msg = spool.tile([128, tu, 512], bf16, tag="msg", bufs=3)
                m4 = msg[:, :, :].rearrange("p t (h c) -> p t h c", c=128)
                nc.scalar.activation(m4, bc(el[:, :, :], 3, 128), AF.Exp)
                nc.vector.tensor_tensor(msg[:, :, :], msg[:, :, :],
                                        hpt[:, :, 0:512], AO.mult)

                pO = p3ps.tile([128, 512], f32, tag="pO")
                pD = p3ps.tile([128, 4], f32, tag="pD")
                for t in range(tu):
                    nc.tensor.matmul(pO[:, :], sg[:, t, :], msg[:, t, :],
                                     start=(t == 0), stop=(t == tu - 1))
                    nc.tensor.matmul(pD[:, :], sg[:, t, :], ex[:, t, :],
                                     start=(t == 0), stop=(t == tu - 1))

                de = apool.tile([128, 4], f32, tag="de")
                nc.vector.tensor_scalar_add(de[:, :], pD[:, :], 1e-16)
                rc = apool.tile([128, 4], f32, tag="rc")
                nc.vector.reciprocal(rc[:, :], de[:, :])

                ab = spool.tile([128, 512], bf16, tag="ab")
                if use_bias:
                    nb = spool.tile([128, 512], f32, tag="nb")
                    for h in range(4):
                        nc.vector.tensor_scalar(
                            nb[:, h * 128:(h + 1) * 128],
                            pO[:, h * 128:(h + 1) * 128],
                            rc[:, h:h + 1], None, AO.mult)
                    nc.vector.tensor_tensor(nb[:, :], nb[:, :], brps[:, :], AO.add)
                    nc.scalar.activation(ab[:, :], nb[:, :], AF.Prelu,
                                         alpha=al_act[:, :])
                else:
                    for h in range(4):
                        nc.scalar.activation(
                            ab[:, h * 128:(h + 1) * 128],
                            pO[:, h * 128:(h + 1) * 128],
                            AF.Prelu, scale=rc[:, h:h + 1], alpha=al_act[:, :])

                nc.tensor.matmul(pc_[:, :], cmks[:, w:w + 1], ab[:, :],
                                 start=(w == 0), stop=(w == NW - 1),
                                 skip_group_check=True)

            # ---- global mean + fc1 (redundant on every core) ----
            with tc.tile_pool(name="p4", bufs=1) as p4:
                gs = p4.tile([1, 512], f32)
                nc.vector.tensor_scalar(gs[:, :], pc_[:, :], 1.0 / N, None, AO.mult)
                nc.sync.dma_start(gsl[:, :], gs[:, :])
                nc.gpsimd.collective_compute(
                    "AllGather", AO.bypass,
                    replica_groups=[[0, 1, 2, 3, 4, 5, 6, 7]],
                    ins=[gsl[:, :]], outs=[gsa[:, :]])
                gtr = p4.tile([128, 4, 8], f32)
                for r in range(8):
                    nc.sync.dma_start(
                        gtr[:, :, r],
                        gsa[r:r + 1, :].rearrange("o (c p) -> (o p) c", p=128))
                gt = p4.tile([128, 4, 2], f32)
                nc.vector.reduce_sum(
                    gt[:, :, :],
                    gtr[:, :, :].rearrange("p c (g k) -> p c g k", k=4),
                    mybir.AxisListType.X)
                gtb = p4.tile([128, 4, 2], bf16)
                nc.vector.tensor_copy(gtb[:, :, :], gt[:, :, :])
                pF = pcps.tile([128, 8], f32, tag="pF")
                for m in range(4):
                    for k in range(4):
                        nc.tensor.matmul(
                            pF[:, m * 2:(m + 1) * 2],
                            f1tb[:, k, m * 128:(m + 1) * 128], gtb[:, k, :],
                            start=(k == 0), stop=(k == 3),
                            skip_group_check=True)
                fo = p4.tile([128, 4, 3], f32)
                for m in range(4):
                    nc.scalar.activation(fo[:, m, 0:2], pF[:, m * 2:(m + 1) * 2],
                                         AF.Prelu, bias=f1bs[:, m:m + 1],
                                         alpha=al_act[:, :])
                nc.vector.tensor_tensor(fo[:, :, 2:3], fo[:, :, 0:1],
                                        fo[:, :, 1:2], AO.subtract)
                nc.sync.dma_start(outp[:, :], fo[:, :, :])

    nc.compile()
    _BUILD_CACHE[key] = nc
    return nc


# --------------------------------------------------------------------------
# entry point
# --------------------------------------------------------------------------

def kernel(x, adj, wt_x, wt_adj, W, att_src, att_dst, bias, fc1_w, fc1_b):
    from concourse.bass_utils import run_bass_kernel_spmd

    x = np.asarray(x, np.float32)
    wt_x = np.asarray(wt_x, np.float32)
    adj = np.asarray(adj)
    wt_adj = np.asarray(wt_adj)
    W = np.asarray(W, np.float32)
    att_src = np.asarray(att_src, np.float32)
    att_dst = np.asarray(att_dst, np.float32)
    bias = np.asarray(bias, np.float32)
    fc1_w = np.asarray(fc1_w, np.float32)
    fc1_b = np.asarray(fc1_b, np.float32)

    pba = _prep_branch(x, adj)
    pbb = _prep_branch(wt_x, wt_adj)
    tu = max(pba["tu"], pbb["tu"])
    pca = _finalize_branch(pba, tu)
    pcb = _finalize_branch(pbb, tu)

    wa_s = np.einsum("fhc,hc->fh", W.reshape(F, HEADS, 128), att_src)
    wa_d = np.einsum("fhc,hc->fh", W.reshape(F, HEADS, 128), att_dst)
    wmat = np.ascontiguousarray(
        np.concatenate([W, wa_s, wa_d], axis=1).astype(np.float32))
    f1br = np.ascontiguousarray(fc1_b.reshape(4, 128).T.astype(np.float32))
    use_bias = bool(np.any(bias != 0.0))

    nc = _build(tu, use_bias)

    in_maps = []
    for core in range(NCORES):
        pc = (pca if core < GROUP else pcb)[core % GROUP]
        m = dict(xt=pc["xt"], wm=wmat, f1t=fc1_w, f1b=f1br,
                 sidx=pc["sidx"], sfp=pc["sfp"], stp=pc["stp"],
                 cmk=pc["cmk"])
        if use_bias:
            m["brp"] = np.ascontiguousarray(
                np.tile(bias[None, :], (128, 1)).astype(np.float32))
        in_maps.append(m)

    trace = bool(int(__import__("os").environ.get("GNN_TRACE", "0")))
    res = run_bass_kernel_spmd(nc, in_maps, core_ids=list(range(NCORES)),
                               trace=trace)
    kernel.last_exec_time_ns = res.exec_time_ns
    kernel.last_res = res
    o = np.asarray(res.results[0]["out"]).reshape(128, 4, 3)
    o2 = o.transpose(1, 0, 2).reshape(512, 3)
    return np.ascontiguousarray(
        np.concatenate([o2[:, 0], o2[:, 1], o2[:, 2]])[None, :]).astype(np.float32)
